# revision 18
# baseline (speedup 1.0000x reference)
"""Trainium2 Bass kernel for nn_MemoryLayer (cross-attention + MLP).

Sharding: data-parallel over (B, S) — 8 cores, each handles 1024 query rows
(batch c//2, sequence half c%2) against its batch's full memory (M=4096).
No collectives; host slices inputs / reassembles outputs.

Per-core pipeline (S_LOC=1024, M=4096, D=512, MLP=1024), bf16 matmuls with
fp32 PSUM accumulation. Algebraic folds cut PE work:
  t0^T = wkq^T x^T + (Wk bq)      where wkq = Wq Wk^T (host-folded)
  qbk  = x (Wq bk) + bq.bk        per-query additive score constant
  scores = t0 m^T (+ qbk)  -> exp (ACT, accum_out rowsum) -> normalize (DVE)
         -> attn out (fp32 DMA) + bf16 cast (GPSIMD) -> DMA xbar transposes
  t1 = m^T attn^T  (PE, slot-major contraction)
  Vbar^T = Wv^T t1 + bv           (V projection applied post-attention:
                                   attn @ (m Wv) == (attn @ m) @ Wv)
  h^T = relu(W1^T Vbar^T + b1);  out = h W2 + b2 (rank-1 ones matmul)

Softmax max-subtraction is skipped: scores are bounded (|s| < ~2 with the
0.02-scaled weights), mathematically identical result.
"""

import os
import numpy as np
import ml_dtypes

import concourse.bass as bass
import concourse.tile as tile
from concourse import bacc, mybir
from concourse.bass_utils import run_bass_kernel_spmd

# Full-problem shapes
B, S, M = 4, 2048, 4096
D = 512          # IN_DIM = MEM_DIM = MODEL_DIM
MLP = 1024
N_CORES = 8
S_LOC = (B * S) // N_CORES  # 1024 query rows per core

NK = D // 128       # 4 contraction chunks
NQ = S_LOC // 128   # 8 query chunks
NS = M // 128       # 32 memory-slot chunks
NM = MLP // 128     # 8 mlp chunks

SCALE = float(1.0 / np.sqrt(np.float32(D)))

F32 = mybir.dt.float32
BF16 = mybir.dt.bfloat16
AF = mybir.ActivationFunctionType
BF = ml_dtypes.bfloat16

_NC_CACHE = None


def _build_kernel():
    nc = bacc.Bacc("TRN2", debug=False, enable_asserts=False, num_devices=N_CORES)

    xT = nc.dram_tensor("xT", [D, S_LOC], BF16, kind="ExternalInput").ap()
    mT = nc.dram_tensor("mT", [D, M], BF16, kind="ExternalInput").ap()
    mN = nc.dram_tensor("mN", [M, D], BF16, kind="ExternalInput").ap()
    wkq = nc.dram_tensor("wkq", [D, D], BF16, kind="ExternalInput").ap()
    wv = nc.dram_tensor("wv", [D, D], BF16, kind="ExternalInput").ap()
    w1 = nc.dram_tensor("w1", [D, MLP], BF16, kind="ExternalInput").ap()
    w2 = nc.dram_tensor("w2", [MLP, D], BF16, kind="ExternalInput").ap()
    wqbk = nc.dram_tensor("wqbk", [128, NK], BF16, kind="ExternalInput").ap()
    bqbk = nc.dram_tensor("bqbk", [128, 1], F32, kind="ExternalInput").ap()
    wkbq = nc.dram_tensor("wkbq", [128, NK], F32, kind="ExternalInput").ap()
    bv_c = nc.dram_tensor("bv_c", [128, NK], F32, kind="ExternalInput").ap()
    b1 = nc.dram_tensor("b1", [128, NM], F32, kind="ExternalInput").ap()
    b2_row = nc.dram_tensor("b2_row", [1, D], BF16, kind="ExternalInput").ap()

    out_d = nc.dram_tensor("out", [S_LOC, D], F32, kind="ExternalOutput").ap()
    attn_d = nc.dram_tensor("attn", [S_LOC, M], F32, kind="ExternalOutput").ap()

    with tile.TileContext(nc) as tc:
        _body(tc, xT, mT, mN, wkq, wv, w1, w2, wqbk, bqbk, wkbq, bv_c, b1,
              b2_row, out_d, attn_d)

    nc.compile()
    return nc


def _body(tc, xT, mT, mN, wkq, wv, w1, w2, wqbk, bqbk, wkbq, bv_c, b1,
          b2_row, out_d, attn_d):
    nc = tc.nc
    X = mybir.AxisListType.X

    from contextlib import ExitStack
    es = ExitStack()
    with es:
        consts = es.enter_context(tc.tile_pool(name="consts", bufs=1))
        actp = es.enter_context(tc.tile_pool(name="actp", bufs=1))
        attnp = es.enter_context(tc.tile_pool(name="attnp", bufs=2))
        bfp = es.enter_context(tc.tile_pool(name="bfp", bufs=2))
        vbp = es.enter_context(tc.tile_pool(name="vbp", bufs=1))
        misc = es.enter_context(tc.tile_pool(name="misc", bufs=4))
        bigps = es.enter_context(tc.tile_pool(name="bigps", bufs=2, space="PSUM"))
        t1psp = es.enter_context(tc.tile_pool(name="t1psp", bufs=1, space="PSUM"))
        projps = es.enter_context(tc.tile_pool(name="projps", bufs=2, space="PSUM"))

        # ---- loads, ordered so phase B / C0 unblock earliest ----
        inpool = tc.tile_pool(name="inp", bufs=1)
        inp = inpool.__enter__()
        xT_sb = inp.tile([128, NK, S_LOC], BF16)
        xT_r = xT.rearrange("(c p) s -> p c s", p=128)
        nc.sync.dma_start(xT_sb[:, :, 0:512], xT_r[:, :, 0:512])
        nc.sync.dma_start(xT_sb[:, :, 512:1024], xT_r[:, :, 512:1024])
        wkq_sb = consts.tile([128, NK, D], BF16)
        nc.scalar.dma_start(wkq_sb[:], wkq.rearrange("(c p) d -> p c d", p=128))
        wqbk_sb = consts.tile([128, NK], BF16)
        nc.scalar.dma_start(wqbk_sb[:], wqbk)
        bqbk_sb = consts.tile([128, 1], F32)
        nc.scalar.dma_start(bqbk_sb[:], bqbk)
        wkbq_sb = consts.tile([128, NK], F32)
        nc.scalar.dma_start(wkbq_sb[:], wkbq)
        mT_sb = actp.tile([128, NK, M], BF16)
        mT_r = mT.rearrange("(c p) s -> p c s", p=128)
        for g in range(4):
            eng = nc.sync if g % 2 == 0 else nc.scalar
            eng.dma_start(mT_sb[:, :, g * 1024:(g + 1) * 1024],
                          mT_r[:, :, g * 1024:(g + 1) * 1024])
        mN_sb = actp.tile([128, NS, D], BF16)
        mN_r = mN.rearrange("(c p) d -> p c d", p=128)
        for g in range(2):
            eng = nc.sync if g % 2 == 0 else nc.scalar
            eng.dma_start(mN_sb[:, g * 16:(g + 1) * 16, :],
                          mN_r[:, g * 16:(g + 1) * 16, :])
        wv_sb = consts.tile([128, NK, D], BF16)
        nc.sync.dma_start(wv_sb[:], wv.rearrange("(c p) d -> p c d", p=128))
        bv_sb = consts.tile([128, NK], F32)
        nc.sync.dma_start(bv_sb[:], bv_c)
        b1_sb = consts.tile([128, NM], F32)
        nc.sync.dma_start(b1_sb[:], b1)
        b2_sb = consts.tile([1, D], BF16)
        nc.sync.dma_start(b2_sb[:], b2_row)
        ones_sb = consts.tile([1, 128], BF16)
        nc.vector.memset(ones_sb[:], 1.0)
        t0T_sb = actp.tile([128, NK, S_LOC], BF16)
        t1_sb = actp.tile([128, NK, S_LOC], BF16)
        qbk_sb = actp.tile([128, NQ], F32)

        # ---- Phase B: t0^T = wkq^T x^T + wkbq ; qbk = x wqbk + bq.bk ----
        for d in range(NK):
            for nh in range(2):
                ps = projps.tile([128, 512], F32, name="ps_t0", tag="ps_small")
                for k in range(NK):
                    nc.tensor.matmul(
                        ps[:],
                        wkq_sb[:, k, d * 128:(d + 1) * 128],
                        xT_sb[:, k, nh * 512:(nh + 1) * 512],
                        start=(k == 0), stop=(k == NK - 1),
                    )
                nc.vector.tensor_scalar_add(
                    t0T_sb[:, d, nh * 512:(nh + 1) * 512], ps[:],
                    wkbq_sb[:, d:d + 1],
                )

        for qc in range(NQ):
            ps = projps.tile([128, 1], F32, name="ps_qbk", tag="ps_small")
            for k in range(NK):
                nc.tensor.matmul(
                    ps[:],
                    xT_sb[:, k, qc * 128:(qc + 1) * 128],
                    wqbk_sb[:, k:k + 1],
                    start=(k == 0), stop=(k == NK - 1),
                )
            # qbk = ps * SCALE + SCALE*(bq.bk)   (bqbk pre-scaled on host)
            nc.vector.tensor_scalar(
                qbk_sb[:, qc:qc + 1], ps[:], SCALE, bqbk_sb[:, 0:1],
                op0=mybir.AluOpType.mult, op1=mybir.AluOpType.add,
            )
        inpool.__exit__(None, None, None)

        # ---- Phase C: scores -> softmax -> attn out + transposed bf16 copy ----
        def emit_qc_compute(qc):
            attn_t = attnp.tile([128, M], F32, name="attn_t")
            rs = misc.tile([128, 4], F32, name="rs")
            for g in range(4):
                ps = bigps.tile([128, 1024], F32, name="ps_s", tag="ps_big")
                for half in range(2):
                    ns = g * 2 + half
                    for k in range(NK):
                        nc.tensor.matmul(
                            ps[:, half * 512:(half + 1) * 512],
                            t0T_sb[:, k, qc * 128:(qc + 1) * 128],
                            mT_sb[:, k, ns * 512:(ns + 1) * 512],
                            start=(k == 0), stop=(k == NK - 1),
                        )
                nc.scalar.activation(
                    attn_t[:, g * 1024:(g + 1) * 1024], ps[:],
                    AF.Exp, scale=SCALE, bias=qbk_sb[:, qc:qc + 1],
                    accum_out=rs[:, g:g + 1],
                )
            rowsum = misc.tile([128, 1], F32, name="rowsum")
            nc.vector.reduce_sum(rowsum[:], rs[:], axis=X)
            recip = misc.tile([128, 1], F32, name="recip")
            nc.vector.reciprocal(recip[:], rowsum[:])
            # normalized bf16 copy (feeds xbar transposes), then fp32 path
            attn_bf = bfp.tile([128, M], BF16, name="attn_bf")
            nc.vector.tensor_scalar_mul(attn_bf[:, 0:M // 2],
                                        attn_t[:, 0:M // 2], recip[:])
            nc.vector.tensor_scalar_mul(attn_bf[:, M // 2:M],
                                        attn_t[:, M // 2:M], recip[:])
            nc.vector.tensor_scalar_mul(attn_t[:], attn_t[:], recip[:])
            nc.sync.dma_start(attn_d[qc * 128:(qc + 1) * 128, :], attn_t[:])
            return attn_bf

        def emit_qc_transpose(qc, attnT, attn_bf):
            # transposes keep to the scalar HWDGE queue so a WAR-blocked
            # transpose never holds up attn stores (which ride nc.sync)
            j = qc % 4
            half_ns = NS // 2
            nc.scalar.dma_start_transpose(
                attnT[:, j, 0:half_ns, :], attn_bf[:, 0:M // 2])
            nc.scalar.dma_start_transpose(
                attnT[:, j, half_ns:NS, :], attn_bf[:, M // 2:M])

        def emit_qc(qc, attnT):
            attn_bf = emit_qc_compute(qc)
            emit_qc_transpose(qc, attnT, attn_bf)

        def emit_t1(qh, attnT):
            # t1 = m^T attn^T (contraction over slots), two d-pair passes
            # through a single 2-bank psum tile
            for dp in range(2):
                t1ps = t1psp.tile([128, 2, 512], F32, name="t1ps", tag="t1ps")
                for sc in range(NS):
                    for dd in range(2):
                        d = dp * 2 + dd
                        nc.tensor.matmul(
                            t1ps[:, dd, :],
                            mN_sb[:, sc, d * 128:(d + 1) * 128],
                            attnT[:, :, sc, :],
                            start=(sc == 0), stop=(sc == NS - 1),
                        )
                for dd in range(2):
                    d = dp * 2 + dd
                    nc.vector.tensor_copy(
                        t1_sb[:, d, qh * 512:(qh + 1) * 512], t1ps[:, dd, :])

        vbT_sb = vbp.tile([128, NK, S_LOC], BF16)
        w1p = es.enter_context(tc.tile_pool(name="w1p", bufs=1))
        w1_sb = w1p.tile([128, NK, MLP], BF16)
        nc.scalar.dma_start(w1_sb[:], w1.rearrange("(c p) d -> p c d", p=128))
        hT_sb = w1p.tile([128, NM, S_LOC], BF16)

        def emit_vbar_mlp(qh):
            # Vbar^T = Wv^T t1 + bv (this query half)
            for d in range(NK):
                ps = projps.tile([128, 512], F32, name="ps_vb", tag="ps_small")
                for k in range(NK):
                    nc.tensor.matmul(
                        ps[:],
                        wv_sb[:, k, d * 128:(d + 1) * 128],
                        t1_sb[:, k, qh * 512:(qh + 1) * 512],
                        start=(k == 0), stop=(k == NK - 1),
                    )
                nc.vector.tensor_scalar_add(
                    vbT_sb[:, d, qh * 512:(qh + 1) * 512], ps[:],
                    bv_sb[:, d:d + 1],
                )
            # h^T = relu(W1^T Vbar^T + b1) (this query half)
            for mc in range(NM):
                ps = projps.tile([128, 512], F32, name="ps_h", tag="ps_small")
                for k in range(NK):
                    nc.tensor.matmul(
                        ps[:],
                        w1_sb[:, k, mc * 128:(mc + 1) * 128],
                        vbT_sb[:, k, qh * 512:(qh + 1) * 512],
                        start=(k == 0), stop=(k == NK - 1),
                    )
                nc.scalar.activation(
                    hT_sb[:, mc, qh * 512:(qh + 1) * 512], ps[:],
                    AF.Relu, bias=b1_sb[:, mc:mc + 1],
                )

        with tc.tile_pool(name="attnT_p", bufs=1) as attnTp:
            attnT0 = attnTp.tile([128, 4, NS, 128], BF16, name="attnT")
            for qc in range(4):
                emit_qc(qc, attnT0)
            attnT1 = attnTp.tile([128, 4, NS, 128], BF16, name="attnT")
            emit_qc(4, attnT1)
            emit_t1(0, attnT0)
            for qc in range(5, 8):
                emit_qc(qc, attnT1)
            emit_vbar_mlp(0)
            emit_t1(1, attnT1)
        emit_vbar_mlp(1)

        # ---- Phase E prep ----
        latep = es.enter_context(tc.tile_pool(name="latep", bufs=1))
        w2_sb = latep.tile([128, NM, D], BF16)
        nc.sync.dma_start(w2_sb[:], w2.rearrange("(c p) d -> p c d", p=128))

        # ---- Phase E: out = h @ W2 + b2 ----
        for qc in range(NQ):
            ps = projps.tile([128, 512], F32, name="ps_o", tag="ps_small")
            for mc in range(NM):
                nc.tensor.matmul(
                    ps[:],
                    hT_sb[:, mc, qc * 128:(qc + 1) * 128],
                    w2_sb[:, mc, :],
                    start=(mc == 0), stop=False,
                )
            nc.tensor.matmul(ps[:], ones_sb[:], b2_sb[:],
                             start=False, stop=True)
            out_sb = misc.tile([128, 512], F32, name="out_sb", bufs=2)
            nc.vector.tensor_copy(out_sb[:], ps[:])
            nc.sync.dma_start(out_d[qc * 128:(qc + 1) * 128, :], out_sb[:])


def _get_nc():
    global _NC_CACHE
    if _NC_CACHE is None:
        _NC_CACHE = _build_kernel()
    return _NC_CACHE


def _make_in_maps(x, m, Wq, bq, Wk, bk, Wv, bv, W1, b1, W2, b2):
    f32 = np.float32

    def bcol(v, nchunk, dtype=f32):
        return np.ascontiguousarray(
            np.asarray(v, f32).reshape(nchunk, 128).T.astype(dtype))

    Wq64 = np.asarray(Wq, f32); Wk64 = np.asarray(Wk, f32)
    wkq_h = (Wq64 @ Wk64.T).astype(BF)                    # [IN, MEM]
    wqbk_h = bcol(Wq64 @ np.asarray(bk, f32), NK, BF)     # [128, NK] bf16
    wkbq_h = bcol(Wk64 @ np.asarray(bq, f32), NK)         # [128, NK] f32
    bqbk_h = np.full((128, 1), SCALE * float(np.asarray(bq, f32)
                                             @ np.asarray(bk, f32)), f32)

    shared = {
        "wkq": np.ascontiguousarray(wkq_h),
        "wv": np.ascontiguousarray(np.asarray(Wv).astype(BF)),
        "w1": np.ascontiguousarray(np.asarray(W1).astype(BF)),
        "w2": np.ascontiguousarray(np.asarray(W2).astype(BF)),
        "wqbk": wqbk_h,
        "bqbk": bqbk_h,
        "wkbq": wkbq_h,
        "bv_c": bcol(bv, NK),
        "b1": bcol(b1, NM),
        "b2_row": np.asarray(b2).astype(BF).reshape(1, D),
    }
    in_maps = []
    for c in range(N_CORES):
        b, h = divmod(c, 2)
        xs = np.asarray(x[b, h * S_LOC:(h + 1) * S_LOC, :])
        ms_bf = np.asarray(m[b]).astype(BF)
        in_maps.append(dict(
            shared,
            xT=np.ascontiguousarray(xs.astype(BF).T),
            mT=np.ascontiguousarray(ms_bf.T),
            mN=np.ascontiguousarray(ms_bf),
        ))
    return in_maps


def kernel(x, m, Wq, bq, Wk, bk, Wv, bv, W1, b1, W2, b2):
    nc = _get_nc()
    in_maps = _make_in_maps(x, m, Wq, bq, Wk, bk, Wv, bv, W1, b1, W2, b2)
    trace = bool(int(os.environ.get("KERNEL_TRACE", "0")))
    try:
        res = run_bass_kernel_spmd(nc, in_maps, core_ids=list(range(N_CORES)),
                                   trace=trace)
    except ModuleNotFoundError:
        trace = False
        res = run_bass_kernel_spmd(nc, in_maps, core_ids=list(range(N_CORES)),
                                   trace=False)
    if trace and res.exec_time_ns is not None:
        print(f"HW exec time: {res.exec_time_ns} ns")
    out = np.empty((B, S, D), np.float32)
    attn = np.empty((B, S, M), np.float32)
    for c, r in enumerate(res.results):
        b, h = divmod(c, 2)
        out[b, h * S_LOC:(h + 1) * S_LOC] = r["out"]
        attn[b, h * S_LOC:(h + 1) * S_LOC] = r["attn"]
    return out, attn
